# revision 2
# baseline (speedup 1.0000x reference)
"""MinGRU cell kernel for Trainium2 (8 NeuronCores, batch-parallel).

Reference computation (per batch b):
    k = x @ Wz.T + bz            # [S, H]
    u = x @ Wh.T + bh            # [S, H]
    z = sigmoid(k);  c = 1 - z = sigmoid(-k)
    g = where(u >= 0, u + 0.5, sigmoid(u)) = max(u + 0.5, sigmoid(u))
    h_t = c_t * h_{t-1} + z_t * g_t,   h_init = g(h_0)
The reference evaluates this scan in log-space; the linear-space recurrence is
a convex combination (c in (0,1), v >= 0) and is numerically tighter in fp32
(3e-6 vs the reference's own 6e-4 max-rel deviation from float64 truth).

Device layout: channels on partitions (768 = 6 x 128), time on the free axis.
  - TensorE: k/u projections, weights stationary ([d,h] tiles), x.T as rhs.
  - ScalarE: the three sigmoids (biases applied via the free per-partition
    bias operand of ACTIVATE).
  - VectorE: g via one fused scalar_tensor_tensor, v = z*g, and the scan via
    the native tensor_tensor_scan (state = c*state + v) chained across chunks.
Each core processes one batch element. Host pre-transposes x / weights and
un-transposes the [B, H, S] device output (numpy view, no copy cost on HW).
"""

import numpy as np

B, S, D, H = 8, 4096, 768, 768
TCH = 512                 # time chunk (matmul moving free dim, fp32 max)
NT = S // TCH             # 8 time chunks
KJ = D // 128             # 6 contraction sub-tiles
HJ = H // 128             # 6 channel tiles

_CACHE = {}


def _build_nc(n_tchunks=NT):
    import concourse.bacc as bacc
    import concourse.mybir as mybir
    import concourse.tile as tile

    fp32 = mybir.dt.float32
    Act = mybir.ActivationFunctionType
    Alu = mybir.AluOpType

    s = n_tchunks * TCH
    nc = bacc.Bacc("TRN2", target_bir_lowering=False, debug=False)

    xt = nc.dram_tensor("xt", [D, s], fp32, kind="ExternalInput").ap()
    wzt = nc.dram_tensor("wzt", [D, H], fp32, kind="ExternalInput").ap()
    wht = nc.dram_tensor("wht", [D, H], fp32, kind="ExternalInput").ap()
    # per-partition scalars, laid out [128, HJ]: col j serves channel tile j
    h0g = nc.dram_tensor("h0g", [128, HJ], fp32, kind="ExternalInput").ap()
    bzn = nc.dram_tensor("bzn", [128, HJ], fp32, kind="ExternalInput").ap()   # -bz
    bzp = nc.dram_tensor("bzp", [128, HJ], fp32, kind="ExternalInput").ap()   # +bz
    bhp = nc.dram_tensor("bhp", [128, HJ], fp32, kind="ExternalInput").ap()   # +bh
    bh5 = nc.dram_tensor("bh5", [128, HJ], fp32, kind="ExternalInput").ap()   # bh+0.5
    ht = nc.dram_tensor("ht", [H, s], fp32, kind="ExternalOutput").ap()

    with tile.TileContext(nc) as tc:
        with (
            tc.tile_pool(name="consts", bufs=1) as consts,
            tc.tile_pool(name="xin", bufs=3) as xin,
            tc.tile_pool(name="temps", bufs=3) as temps,
            tc.tile_pool(name="hout", bufs=3) as hout,
            tc.tile_pool(name="psum", bufs=3, space="PSUM") as psum,
        ):
            wz_sb = consts.tile([128, KJ, H], fp32, tag="wz")
            wh_sb = consts.tile([128, KJ, H], fp32, tag="wh")
            nc.sync.dma_start(wz_sb[:], wzt.rearrange("(a p) h -> p a h", p=128))
            nc.sync.dma_start(wh_sb[:], wht.rearrange("(a p) h -> p a h", p=128))
            sc_sb = consts.tile([128, 5, HJ], fp32, tag="scalars")
            for idx, src in enumerate((h0g, bzn, bzp, bhp, bh5)):
                nc.sync.dma_start(sc_sb[:, idx], src)

            xt_r = xt.rearrange("(a p) t -> p a t", p=128)
            ht_r = ht.rearrange("(j p) t -> j p t", p=128)

            h_prev = [None] * HJ
            for i in range(n_tchunks):
                x_sb = xin.tile([128, KJ, TCH], fp32, tag="x")
                nc.sync.dma_start(x_sb[:], xt_r[:, :, i * TCH:(i + 1) * TCH])
                for j in range(HJ):
                    hsl = slice(j * 128, (j + 1) * 128)
                    pk = psum.tile([128, TCH], fp32, tag="pk")
                    pu = psum.tile([128, TCH], fp32, tag="pu")
                    for a in range(KJ):
                        nc.tensor.matmul(pk, wz_sb[:, a, hsl], x_sb[:, a],
                                         start=(a == 0), stop=(a == KJ - 1))
                    for a in range(KJ):
                        nc.tensor.matmul(pu, wh_sb[:, a, hsl], x_sb[:, a],
                                         start=(a == 0), stop=(a == KJ - 1))
                    c_sb = temps.tile([128, TCH], fp32, tag="c")
                    z_sb = temps.tile([128, TCH], fp32, tag="z")
                    sg_sb = temps.tile([128, TCH], fp32, tag="sg")
                    g_sb = temps.tile([128, TCH], fp32, tag="g")
                    v_sb = temps.tile([128, TCH], fp32, tag="v")
                    nc.scalar.activation(c_sb[:], pk[:], Act.Sigmoid,
                                         bias=sc_sb[:, 1, j:j + 1], scale=-1.0)
                    nc.scalar.activation(z_sb[:], pk[:], Act.Sigmoid,
                                         bias=sc_sb[:, 2, j:j + 1])
                    nc.scalar.activation(sg_sb[:], pu[:], Act.Sigmoid,
                                         bias=sc_sb[:, 3, j:j + 1])
                    nc.vector.scalar_tensor_tensor(g_sb[:], pu[:],
                                                   sc_sb[:, 4, j:j + 1], sg_sb[:],
                                                   op0=Alu.add, op1=Alu.max)
                    nc.vector.tensor_mul(v_sb[:], z_sb[:], g_sb[:])
                    h_sb = hout.tile([128, TCH], fp32, tag=f"h{j}")
                    init = sc_sb[:, 0, j:j + 1] if i == 0 else h_prev[j][:, TCH - 1:TCH]
                    nc.vector.tensor_tensor_scan(h_sb[:], c_sb[:], v_sb[:], init,
                                                 op0=Alu.mult, op1=Alu.add)
                    h_prev[j] = h_sb
                    nc.sync.dma_start(ht_r[j, :, i * TCH:(i + 1) * TCH], h_sb[:])
    nc.compile()
    return nc


def _get_nc(n_tchunks=NT):
    if n_tchunks not in _CACHE:
        _CACHE[n_tchunks] = _build_nc(n_tchunks)
    return _CACHE[n_tchunks]


def _sigmoid(x):
    return 1.0 / (1.0 + np.exp(-x))


def _host_inputs(x, h_0, Wz, bz, Wh, bh):
    """Build the per-core input maps (host-side layout only)."""
    x = np.asarray(x, dtype=np.float32)
    h_0 = np.asarray(h_0, dtype=np.float32)
    Wz = np.asarray(Wz, dtype=np.float32)
    Wh = np.asarray(Wh, dtype=np.float32)
    bz = np.asarray(bz, dtype=np.float32)
    bh = np.asarray(bh, dtype=np.float32)
    b, s, d = x.shape
    h = Wz.shape[0]
    xt = np.ascontiguousarray(np.swapaxes(x, 1, 2))          # [B, D, S]
    wzt = np.ascontiguousarray(Wz.T)                         # [D, H]
    wht = np.ascontiguousarray(Wh.T)
    h0 = h_0.reshape(b, h)
    h0g = np.maximum(h0 + 0.5, _sigmoid(h0)).astype(np.float32)   # g(h_0)

    def cols(vec):  # [H] -> [128, HJ] with arr[p, j] = vec[j*128+p]
        return np.ascontiguousarray(vec.reshape(h // 128, 128).T.astype(np.float32))

    in_maps = []
    for bi in range(b):
        in_maps.append({
            "xt": xt[bi],
            "wzt": wzt,
            "wht": wht,
            "h0g": cols(h0g[bi]),
            "bzn": cols(-bz),
            "bzp": cols(bz),
            "bhp": cols(bh),
            "bh5": cols(bh + 0.5),
        })
    return in_maps


def run_device(x, h_0, Wz, bz, Wh, bh, trace=False, **trace_kwargs):
    """Run on the 8 NeuronCores; returns (out [B,S,H], BassKernelResults)."""
    from concourse.bass_utils import run_bass_kernel_spmd

    in_maps = _host_inputs(x, h_0, Wz, bz, Wh, bh)
    nc = _get_nc()
    res = run_bass_kernel_spmd(nc, in_maps, core_ids=list(range(len(in_maps))),
                               trace=trace, **trace_kwargs)
    out_t = np.stack([r["ht"] for r in res.results])         # [B, H, S]
    out = np.swapaxes(out_t, 1, 2)                           # [B, S, H] view
    return out, res


def kernel(x, h_0, Wz, bz, Wh, bh):
    out, _ = run_device(x, h_0, Wz, bz, Wh, bh)
    return out


# revision 3
# speedup vs baseline: 3.1161x; 3.1161x over previous
"""MinGRU cell kernel for Trainium2 (8 NeuronCores, batch-parallel).

Reference computation (per batch b):
    k = x @ Wz.T + bz            # [S, H]
    u = x @ Wh.T + bh            # [S, H]
    z = sigmoid(k);  c = 1 - z = sigmoid(-k)
    g = where(u >= 0, u + 0.5, sigmoid(u)) = max(u + 0.5, sigmoid(u))
    h_t = c_t * h_{t-1} + z_t * g_t,   h_init = g(h_0)
The reference evaluates this scan in log-space; the linear-space recurrence is
a convex combination (c in (0,1), v >= 0) and is numerically tighter in fp32
(3e-6 vs the reference's own 6e-4 max-rel deviation from float64 truth).

Device layout: channels on partitions (768 = 6 x 128), time on the free axis.
  - TensorE: k/u projections, weights stationary ([d,h] tiles), x.T as rhs.
  - ScalarE: the three sigmoids (biases applied via the free per-partition
    bias operand of ACTIVATE).
  - VectorE: g via one fused scalar_tensor_tensor, v = z*g, and the scan via
    the native tensor_tensor_scan (state = c*state + v) chained across chunks.
Each core processes one batch element. Host pre-transposes x / weights and
un-transposes the [B, H, S] device output (numpy view, no copy cost on HW).
"""

import numpy as np

B, S, D, H = 8, 4096, 768, 768
TCH = 512                 # time chunk (matmul moving free dim, fp32 max)
NT = S // TCH             # 8 time chunks
KJ = D // 128             # 6 contraction sub-tiles
HJ = H // 128             # 6 channel tiles

_CACHE = {}


def _build_nc(n_tchunks=NT):
    import concourse.bacc as bacc
    import concourse.mybir as mybir
    import concourse.tile as tile

    fp32 = mybir.dt.float32
    f32r = mybir.dt.float32r   # fp32 bits; single-pass matmul (~4x fp32 rate)
    Act = mybir.ActivationFunctionType
    Alu = mybir.AluOpType

    s = n_tchunks * TCH
    nc = bacc.Bacc("TRN2", target_bir_lowering=False, debug=False)

    xt = nc.dram_tensor("xt", [D, s], f32r, kind="ExternalInput").ap()
    wzt = nc.dram_tensor("wzt", [D, H], f32r, kind="ExternalInput").ap()
    wht = nc.dram_tensor("wht", [D, H], f32r, kind="ExternalInput").ap()
    # per-partition scalars, laid out [128, HJ]: col j serves channel tile j
    h0g = nc.dram_tensor("h0g", [128, HJ], fp32, kind="ExternalInput").ap()
    bzn = nc.dram_tensor("bzn", [128, HJ], fp32, kind="ExternalInput").ap()   # -bz
    bzp = nc.dram_tensor("bzp", [128, HJ], fp32, kind="ExternalInput").ap()   # +bz
    bhp = nc.dram_tensor("bhp", [128, HJ], fp32, kind="ExternalInput").ap()   # +bh
    bh5 = nc.dram_tensor("bh5", [128, HJ], fp32, kind="ExternalInput").ap()   # bh+0.5
    ht = nc.dram_tensor("ht", [H, s], fp32, kind="ExternalOutput").ap()

    with tile.TileContext(nc) as tc:
        with (
            tc.tile_pool(name="consts", bufs=1) as consts,
            tc.tile_pool(name="xin", bufs=3) as xin,
            tc.tile_pool(name="temps", bufs=3) as temps,
            tc.tile_pool(name="hout", bufs=3) as hout,
            tc.tile_pool(name="psum", bufs=3, space="PSUM") as psum,
        ):
            wz_sb = consts.tile([128, KJ, H], f32r, tag="wz")
            wh_sb = consts.tile([128, KJ, H], f32r, tag="wh")
            nc.sync.dma_start(wz_sb[:], wzt.rearrange("(a p) h -> p a h", p=128))
            nc.sync.dma_start(wh_sb[:], wht.rearrange("(a p) h -> p a h", p=128))
            sc_sb = consts.tile([128, 5, HJ], fp32, tag="scalars")
            for idx, src in enumerate((h0g, bzn, bzp, bhp, bh5)):
                nc.sync.dma_start(sc_sb[:, idx], src)

            xt_r = xt.rearrange("(a p) t -> p a t", p=128)
            ht_r = ht.rearrange("(j p) t -> j p t", p=128)

            h_prev = [None] * HJ
            for i in range(n_tchunks):
                x_sb = xin.tile([128, KJ, TCH], f32r, tag="x")
                nc.sync.dma_start(x_sb[:], xt_r[:, :, i * TCH:(i + 1) * TCH])
                for j in range(HJ):
                    hsl = slice(j * 128, (j + 1) * 128)
                    pk = psum.tile([128, TCH], fp32, tag="pk")
                    pu = psum.tile([128, TCH], fp32, tag="pu")
                    for a in range(KJ):
                        nc.tensor.matmul(pk, wz_sb[:, a, hsl], x_sb[:, a],
                                         start=(a == 0), stop=(a == KJ - 1))
                    for a in range(KJ):
                        nc.tensor.matmul(pu, wh_sb[:, a, hsl], x_sb[:, a],
                                         start=(a == 0), stop=(a == KJ - 1))
                    c_sb = temps.tile([128, TCH], fp32, tag="c")
                    z_sb = temps.tile([128, TCH], fp32, tag="z")
                    sg_sb = temps.tile([128, TCH], fp32, tag="sg")
                    g_sb = temps.tile([128, TCH], fp32, tag="g")
                    v_sb = temps.tile([128, TCH], fp32, tag="v")
                    nc.scalar.activation(c_sb[:], pk[:], Act.Sigmoid,
                                         bias=sc_sb[:, 1, j:j + 1], scale=-1.0)
                    nc.scalar.activation(z_sb[:], pk[:], Act.Sigmoid,
                                         bias=sc_sb[:, 2, j:j + 1])
                    nc.scalar.activation(sg_sb[:], pu[:], Act.Sigmoid,
                                         bias=sc_sb[:, 3, j:j + 1])
                    nc.vector.scalar_tensor_tensor(g_sb[:], pu[:],
                                                   sc_sb[:, 4, j:j + 1], sg_sb[:],
                                                   op0=Alu.add, op1=Alu.max)
                    nc.vector.tensor_mul(v_sb[:], z_sb[:], g_sb[:])
                    h_sb = hout.tile([128, TCH], fp32, tag=f"h{j}")
                    init = sc_sb[:, 0, j:j + 1] if i == 0 else h_prev[j][:, TCH - 1:TCH]
                    nc.vector.tensor_tensor_scan(h_sb[:], c_sb[:], v_sb[:], init,
                                                 op0=Alu.mult, op1=Alu.add)
                    h_prev[j] = h_sb
                    nc.sync.dma_start(ht_r[j, :, i * TCH:(i + 1) * TCH], h_sb[:])
    nc.compile()
    return nc


def _get_nc(n_tchunks=NT):
    if n_tchunks not in _CACHE:
        _CACHE[n_tchunks] = _build_nc(n_tchunks)
    return _CACHE[n_tchunks]


def _sigmoid(x):
    return 1.0 / (1.0 + np.exp(-x))


def _host_inputs(x, h_0, Wz, bz, Wh, bh):
    """Build the per-core input maps (host-side layout only)."""
    x = np.asarray(x, dtype=np.float32)
    h_0 = np.asarray(h_0, dtype=np.float32)
    Wz = np.asarray(Wz, dtype=np.float32)
    Wh = np.asarray(Wh, dtype=np.float32)
    bz = np.asarray(bz, dtype=np.float32)
    bh = np.asarray(bh, dtype=np.float32)
    b, s, d = x.shape
    h = Wz.shape[0]
    xt = np.ascontiguousarray(np.swapaxes(x, 1, 2))          # [B, D, S]
    wzt = np.ascontiguousarray(Wz.T)                         # [D, H]
    wht = np.ascontiguousarray(Wh.T)
    h0 = h_0.reshape(b, h)
    h0g = np.maximum(h0 + 0.5, _sigmoid(h0)).astype(np.float32)   # g(h_0)

    def cols(vec):  # [H] -> [128, HJ] with arr[p, j] = vec[j*128+p]
        return np.ascontiguousarray(vec.reshape(h // 128, 128).T.astype(np.float32))

    in_maps = []
    for bi in range(b):
        in_maps.append({
            "xt": xt[bi],
            "wzt": wzt,
            "wht": wht,
            "h0g": cols(h0g[bi]),
            "bzn": cols(-bz),
            "bzp": cols(bz),
            "bhp": cols(bh),
            "bh5": cols(bh + 0.5),
        })
    return in_maps


def run_device(x, h_0, Wz, bz, Wh, bh, trace=False, **trace_kwargs):
    """Run on the 8 NeuronCores; returns (out [B,S,H], BassKernelResults)."""
    from concourse.bass_utils import run_bass_kernel_spmd

    in_maps = _host_inputs(x, h_0, Wz, bz, Wh, bh)
    nc = _get_nc()
    res = run_bass_kernel_spmd(nc, in_maps, core_ids=list(range(len(in_maps))),
                               trace=trace, **trace_kwargs)
    out_t = np.stack([r["ht"] for r in res.results])         # [B, H, S]
    out = np.swapaxes(out_t, 1, 2)                           # [B, S, H] view
    return out, res


def kernel(x, h_0, Wz, bz, Wh, bh):
    out, _ = run_device(x, h_0, Wz, bz, Wh, bh)
    return out
